# revision 4
# baseline (speedup 1.0000x reference)
"""Trainium2 Bass kernel for a 2-layer GraphConv block (PyG GraphConv, aggr=add):
    h1  = leaky_relu(segsum(x[src], dst) @ W1_rel.T + b1 + x @ W1_root.T)
    out = leaky_relu(segsum(h1[src], dst) @ W2_rel.T + b2 + h1 @ W2_root.T + x)

Self-contained: takes full inputs, shards nodes across 8 NeuronCores internally,
runs one SPMD Bass program (gather/scatter-add DGE ops + PE matmuls + AllGather
halo exchange), and returns the full output.
"""
import sys

sys.path.insert(0, '/opt/trn_rl_repo')

import numpy as np

N = 100000
D = 64
NCORES = 8
NPART = N // NCORES            # 12500
NP = 12544                     # 98 * 128, padded part size
NCHUNK = NP // 128             # 98
DUMP = NP                      # first dump row in agg
G_CHUNK = 512                  # idxs per gather/scatter call; known-good on HW
AGG_ROWS = NP + G_CHUNK        # 14080; rows >= NP are per-call-unique dump rows
SCRATCH = 16384                # dynamic_dma_scratch_size (default; ring=1024 descs)
NEG_SLOPE = 0.01


def _round128(n):
    return ((n + 127) // 128) * 128


def _make_plan(src, dst):
    """Build the uniform SPMD call structure + per-core index streams.

    Edge stream order per core: by (q=src part, r=rank within (dst,q) group, dst).
    Per-(q,r) batch length = max over cores, rounded up to 128.
    Returns (plan, gstreams, sstreams):
      plan = dict(L, gcalls=[(q, start, len)], scalls=[(start, len)])
      gstreams/sstreams: int16 [NCORES, L] (gather idx into part-q tensor / agg row)
    """
    per_core = []
    maxr = 0
    for p in range(NCORES):
        sel = (dst >= p * NPART) & (dst < (p + 1) * NPART)
        s = src[sel]
        d = (dst[sel] - p * NPART).astype(np.int64)
        q = s // NPART
        sl = (s - q * NPART).astype(np.int64)
        o1 = np.lexsort((d, q))
        q, d, sl = q[o1], d[o1], sl[o1]
        key = q * NPART + d
        newgrp = np.r_[True, key[1:] != key[:-1]] if len(key) else np.zeros(0, bool)
        gid = np.cumsum(newgrp) - 1
        starts = np.flatnonzero(newgrp)
        r = np.arange(len(key)) - starts[gid] if len(key) else np.zeros(0, np.int64)
        maxr = max(maxr, int(r.max()) + 1 if len(r) else 0)
        per_core.append((q, r, d, sl))

    # counts[p, q, r]
    counts = np.zeros((NCORES, NCORES, maxr), np.int64)
    for p, (q, r, d, sl) in enumerate(per_core):
        np.add.at(counts, (p, q, r), 1)
    batch_len = np.zeros((NCORES, maxr), np.int64)
    for qq in range(NCORES):
        for rr in range(maxr):
            m = counts[:, qq, rr].max()
            if m > 0:
                batch_len[qq, rr] = _round128(m)

    # stream layout: q-major, r ascending
    batches = []  # (q, r, start, len)
    pos = 0
    for qq in range(NCORES):
        for rr in range(maxr):
            blen = int(batch_len[qq, rr])
            if blen:
                batches.append((qq, rr, pos, blen))
                pos += blen
    L = pos

    # gather calls: cut q-runs at G_CHUNK
    gcalls = []
    for qq in range(NCORES):
        qb = [b for b in batches if b[0] == qq]
        if not qb:
            continue
        q0, q1 = qb[0][2], qb[-1][2] + qb[-1][3]
        a = q0
        while a < q1:
            ln = min(G_CHUNK, q1 - a)
            gcalls.append((qq, a, ln))
            a += ln

    # scatter calls: breakpoints at batch starts + gcall starts, chop at G_CHUNK
    bks = sorted({b[2] for b in batches} | {g[1] for g in gcalls} | {L})
    scalls = []
    for i in range(len(bks) - 1):
        a, b = bks[i], bks[i + 1]
        while a < b:
            ln = min(G_CHUNK, b - a)
            scalls.append((a, ln))
            a += ln

    # per-core streams
    gstreams = np.zeros((NCORES, L), np.int16)
    sstreams = np.zeros((NCORES, L), np.int16)
    for p, (q, r, d, sl) in enumerate(per_core):
        gs = np.zeros(L, np.int64)
        ss = np.full(L, -1, np.int64)
        # edges of (q, r) batch placed at batch start, in d order (lexsort gives d asc
        # within (q, dst) groups -> within (q, r) also d asc)
        o2 = np.lexsort((d, r, q))
        q2, r2, d2, sl2 = q[o2], r[o2], d[o2], sl[o2]
        bstart = {(qq, rr): st for (qq, rr, st, ln) in batches}
        # offsets within each (q,r) batch: edges are sorted by (q,r,d); rank within
        # batch = position - first position of that batch
        key2 = q2 * maxr + r2
        nb = np.r_[True, key2[1:] != key2[:-1]] if len(key2) else np.zeros(0, bool)
        gid2 = np.cumsum(nb) - 1
        st2 = np.flatnonzero(nb)
        off = np.arange(len(key2)) - st2[gid2] if len(key2) else np.zeros(0, np.int64)
        base = np.array([bstart[(int(qq), int(rr))] for qq, rr in
                         zip(q2[st2], r2[st2])], np.int64) if len(st2) else np.zeros(0, np.int64)
        posn = base[gid2] + off
        gs[posn] = sl2
        ss[posn] = d2
        # pads: scatter -> unique dump row per scall
        for (a, ln) in scalls:
            seg = ss[a:a + ln]
            pad = seg < 0
            seg[pad] = DUMP + np.flatnonzero(pad)
        gstreams[p] = gs.astype(np.int16)
        sstreams[p] = ss.astype(np.int16)

    plan = dict(L=L, gcalls=gcalls, scalls=scalls)
    return plan, gstreams, sstreams


def _wrap_stream(a):
    """[L] int16 -> [128, L//16] wrapped (idx i at [i%16, i//16]) replicated 8x."""
    L = len(a)
    assert L % 16 == 0
    w = a.reshape(L // 16, 16).T  # [16, cols]
    return np.tile(w, (8, 1)).copy()


def _build_nc(plan):
    from concourse import tile, mybir, masks
    import concourse.bacc as bacc

    L = plan["L"]
    cols = L // 16
    f32 = mybir.dt.float32
    i16 = mybir.dt.int16

    nc = bacc.Bacc(None, target_bir_lowering=False, num_devices=NCORES,
                   dynamic_dma_scratch_size=SCRATCH)

    x_parts = [nc.declare_dram_parameter(f"x_part{q}", [NP, D], f32, isOutput=False)
               for q in range(NCORES)]
    xT_in = nc.declare_dram_parameter("xT", [D, NP], f32, isOutput=False)
    w_ins = {}
    for nm in ["W1relT", "W1rootT", "W2relT", "W2rootT"]:
        w_ins[nm] = nc.declare_dram_parameter(nm, [D, D], f32, isOutput=False)
    b_ins = {nm: nc.declare_dram_parameter(nm, [1, D], f32, isOutput=False)
             for nm in ["b1", "b2"]}
    gidx_in = nc.declare_dram_parameter("gidx", [128, cols], i16, isOutput=False)
    sidx_in = nc.declare_dram_parameter("sidx", [128, cols], i16, isOutput=False)
    y_out = nc.declare_dram_parameter("y", [NP, D], f32, isOutput=True)

    agg_a = nc.dram_tensor("agg_a", [AGG_ROWS, D], f32)
    agg_b = nc.dram_tensor("agg_b", [AGG_ROWS, D], f32)
    h1_bounce = nc.dram_tensor("h1_bounce", [NP, D], f32)
    h_full = nc.dram_tensor("h_full", [NCORES * NP, D], f32, addr_space="Shared")

    with tile.TileContext(nc) as tc:
        with (
            tc.tile_pool(name="const", bufs=1) as cpool,
            tc.tile_pool(name="idx", bufs=1) as ipool,
            tc.tile_pool(name="gbuf", bufs=4) as gpool,
            tc.tile_pool(name="mm", bufs=3) as mpool,
            tc.tile_pool(name="psum", bufs=2, space="PSUM") as ppool,
        ):
            # ---- constants ----
            ident = cpool.tile([128, 128], f32)
            masks.make_identity(nc, ident[:])
            ones1 = cpool.tile([1, 128], f32)
            nc.gpsimd.memset(ones1[:], 1.0)
            wt = {}
            for nm, t_in in w_ins.items():
                t = cpool.tile([D, D], f32, tag=nm)
                nc.sync.dma_start(t[:], t_in[:])
                wt[nm] = t
            bt = {}
            for nm, t_in in b_ins.items():
                t = cpool.tile([1, D], f32, tag=nm)
                nc.sync.dma_start(t[:], t_in[:])
                bt[nm] = t

            # ---- zero both agg buffers ----
            ztile = cpool.tile([128, D], f32)
            nc.gpsimd.memset(ztile[:], 0.0)
            for agg in (agg_a, agg_b):
                for a in range(0, AGG_ROWS, 128):
                    nc.sync.dma_start(agg[a:a + 128, :], ztile[:])

            # ---- index streams (resident; reused by both layers) ----
            gidx = ipool.tile([128, cols], i16)
            sidx = ipool.tile([128, cols], i16)
            for a in range(0, cols, 2048):
                n = min(2048, cols - a)
                nc.sync.dma_start(gidx[:, a:a + n], gidx_in[:, a:a + n])
                nc.sync.dma_start(sidx[:, a:a + n], sidx_in[:, a:a + n])

            # ---- gather + scatter-add layer ----
            def gs_layer(src_aps, agg):
                for (q, gstart, glen) in plan["gcalls"]:
                    rows = glen // 128
                    gb = gpool.tile([128, G_CHUNK // 128, D], f32, tag="gb")
                    nc.gpsimd.dma_gather(
                        gb[:, :rows, :], src_aps[q], gidx[:, gstart // 16:(gstart + glen) // 16],
                        glen, glen, D)
                    for (sstart, slen) in plan["scalls"]:
                        if sstart < gstart or sstart >= gstart + glen:
                            continue
                        a = (sstart - gstart) // 128
                        b = a + slen // 128
                        nc.gpsimd.dma_scatter_add(
                            agg[:], gb[:, a:b, :],
                            sidx[:, sstart // 16:(sstart + slen) // 16],
                            slen, slen, D)

            # ---- dense phase: h = lrelu(aggT.T@Wrel + rootT.T@Wroot [+ xT.T] + b) ----
            def dense_layer(agg, w_rel, w_root, bias, root_rows, residual, out_rows):
                for c in range(NCHUNK):
                    r0 = c * 128
                    aggc = mpool.tile([128, D], f32, tag="aggc")
                    nc.sync.dma_start(aggc[:], agg[r0:r0 + 128, :])
                    ps_t = ppool.tile([D, 128], f32, tag="ps_t")
                    nc.tensor.transpose(ps_t[:], aggc[:], ident[:])
                    aT = mpool.tile([D, 128], f32, tag="aT")
                    nc.vector.tensor_copy(aT[:], ps_t[:])

                    if root_rows is None:
                        rT = mpool.tile([D, 128], f32, tag="rT")
                        nc.sync.dma_start(rT[:], xT_in[:, r0:r0 + 128])
                    else:
                        hc = mpool.tile([128, D], f32, tag="hc")
                        nc.sync.dma_start(hc[:], root_rows[r0:r0 + 128, :])
                        ps_h = ppool.tile([D, 128], f32, tag="ps_t")
                        nc.tensor.transpose(ps_h[:], hc[:], ident[:])
                        rT = mpool.tile([D, 128], f32, tag="rT")
                        nc.vector.tensor_copy(rT[:], ps_h[:])

                    po = ppool.tile([128, D], f32, tag="po")
                    nc.tensor.matmul(po[:], aT[:], w_rel[:], start=True, stop=False)
                    nc.tensor.matmul(po[:], rT[:], w_root[:], start=False, stop=False)
                    if residual:
                        xTc = mpool.tile([D, 128], f32, tag="xTc")
                        nc.sync.dma_start(xTc[:], xT_in[:, r0:r0 + 128])
                        nc.tensor.matmul(po[:], xTc[:], ident[:D, :D],
                                         start=False, stop=False)
                    nc.tensor.matmul(po[:], ones1[:], bias[:], start=False, stop=True)

                    tmp = mpool.tile([128, D], f32, tag="tmp")
                    nc.vector.tensor_scalar_mul(tmp[:], po[:], NEG_SLOPE)
                    hrow = mpool.tile([128, D], f32, tag="hrow")
                    nc.vector.tensor_max(hrow[:], po[:], tmp[:])
                    nc.sync.dma_start(out_rows[r0:r0 + 128, :], hrow[:])

            # ================= layer 1 =================
            gs_layer([xp[:] for xp in x_parts], agg_a)
            dense_layer(agg_a, wt["W1relT"], wt["W1rootT"], bt["b1"],
                        None, False, h1_bounce)

            # ================= halo exchange =================
            nc.gpsimd.collective_compute(
                "AllGather", mybir.AluOpType.bypass,
                replica_groups=[list(range(NCORES))],
                ins=[h1_bounce[:].opt()], outs=[h_full[:].opt()])

            # ================= layer 2 =================
            gs_layer([h_full[q * NP:(q + 1) * NP, :] for q in range(NCORES)], agg_b)
            dense_layer(agg_b, wt["W2relT"], wt["W2rootT"], bt["b2"],
                        h1_bounce, True, y_out)

    nc.compile()
    return nc


def _prep_inputs(x, edge_index, W1_rel, b1, W1_root, W2_rel, b2, W2_root):
    src = np.asarray(edge_index[0]).astype(np.int64)
    dst = np.asarray(edge_index[1]).astype(np.int64)
    plan, gstreams, sstreams = _make_plan(src, dst)

    x = np.asarray(x, np.float32)
    xp_all = []
    for q in range(NCORES):
        xp = np.zeros((NP, D), np.float32)
        xp[:NPART] = x[q * NPART:(q + 1) * NPART]
        xp_all.append(xp)

    common = {f"x_part{q}": xp_all[q] for q in range(NCORES)}
    common["W1relT"] = np.ascontiguousarray(np.asarray(W1_rel, np.float32).T)
    common["W1rootT"] = np.ascontiguousarray(np.asarray(W1_root, np.float32).T)
    common["W2relT"] = np.ascontiguousarray(np.asarray(W2_rel, np.float32).T)
    common["W2rootT"] = np.ascontiguousarray(np.asarray(W2_root, np.float32).T)
    common["b1"] = np.asarray(b1, np.float32).reshape(1, D)
    common["b2"] = np.asarray(b2, np.float32).reshape(1, D)

    in_maps = []
    for p in range(NCORES):
        m = dict(common)
        m["xT"] = np.ascontiguousarray(xp_all[p].T)
        m["gidx"] = _wrap_stream(gstreams[p])
        m["sidx"] = _wrap_stream(sstreams[p])
        in_maps.append(m)
    return plan, in_maps


def kernel(x, edge_index, W1_rel, b1, W1_root, W2_rel, b2, W2_root):
    from concourse import bass_utils

    plan, in_maps = _prep_inputs(x, edge_index, W1_rel, b1, W1_root,
                                 W2_rel, b2, W2_root)
    nc = _build_nc(plan)
    res = bass_utils.run_bass_kernel_spmd(nc, in_maps, core_ids=list(range(NCORES)))
    out = np.concatenate([res.results[p]["y"][:NPART] for p in range(NCORES)], 0)
    return out.astype(np.float32)


if __name__ == "__main__":
    # quick host-side plan self-check in numpy (no device)
    rng = np.random.default_rng(0)
    E = 200000
    src = rng.integers(0, N, E)
    dst = rng.integers(0, N, E)
    plan, gstreams, sstreams = _make_plan(src, dst)
    print(f"L={plan['L']} gcalls={len(plan['gcalls'])} scalls={len(plan['scalls'])}")
    # emulate per-core layer-1 aggregation and compare against direct segment sum
    x = rng.normal(size=(N, D)).astype(np.float32)
    for p in range(2):
        agg = np.zeros((AGG_ROWS, D), np.float64)
        gs, ss = gstreams[p].astype(np.int64), sstreams[p].astype(np.int64)
        for (q, a, ln) in plan["gcalls"]:
            xq = np.zeros((NP, D), np.float32)
            xq[:NPART] = x[q * NPART:(q + 1) * NPART]
            g = xq[gs[a:a + ln]]
            for (sa, sl) in plan["scalls"]:
                if sa < a or sa >= a + ln:
                    continue
                seg = ss[sa:sa + sl]
                assert len(np.unique(seg)) == len(seg), "dup dst in scall!"
                np.add.at(agg, seg, g[sa - a:sa - a + sl])
        sel = (dst >= p * NPART) & (dst < (p + 1) * NPART)
        ref = np.zeros((NPART, D), np.float64)
        np.add.at(ref, dst[sel] - p * NPART, x[src[sel]])
        err = np.abs(agg[:NPART] - ref).max()
        print(f"core {p}: plan-emulated agg err {err:.3e}")
